# revision 36
# baseline (speedup 1.0000x reference)
"""NeuTraLAD loss kernel for Trainium2, 8-core data parallel.

Shapes (hardcoded): x [16384, 512], K=11 transforms of 3x[512,512] MLPs,
shared 3-layer encoder + LayerNorm, cosine-sim contrastive loss -> [16384].

Strategy: shard batch across 8 cores (2048 rows each, 4 tiles of 512).
- Transform L3 and encoder L1 are both linear pre-gelu, so they are FUSED
  host-side (W3f = tW3 @ eW1), dropping one of six layers entirely.
- The remaining 5 matmul layers run feature-major in fp8 e4m3 with
  DoubleRow perf mode (2 contraction blocks per matmul); weights are
  scaled x256 into fp8's normal range and de-scaled for free via the ACT
  scale port. Gelus drain merged [128,1024] PSUM pairs (biases are zero
  on the fast path, checked at runtime) to halve ACT dispatch overhead.
- The last encoder layer stays bf16 for precision and swaps
  stationary/moving operands to emit z3 SAMPLE-major ([128 samples, 512
  feats] per block), so row sums/sumsq ride the ACT accumulator during
  the PSUM->SBUF copy, and the 66 per-sample cosine dots are single DVE
  scalar_tensor_tensor+accum passes (NOTE: the dedicated
  tensor_tensor_reduce ISA op faults trn2 hardware here).
- Pair dots fire incrementally as each view's encoding completes, so the
  DVE chews on them while the PE runs the next view's layers. The
  logsumexp tail runs per tile: PE-transpose of the [128,66] dot block,
  batched exp, one [66->11] selection matmul for the denominators, ln,
  and two ones-matmuls for the final sum.

Math shortcuts (exact): with ln_g==1, ln_b==0 and all biases zero
(always true for this problem's inputs; checked at runtime with a numpy
fallback otherwise), LN followed by cosine normalization collapses to
zn = (z3-mean)/||z3-mean|| — the LN eps and rstd cancel. Dots are taken
on RAW z3 and mean-centered algebraically via
dot(za-ua, zb-ub) = dot(za, zb) - 512*ua*ub, then scaled by
rn = rsqrt(max(512*var, 1e-16)), identical to the reference clamp
(max(n,eps)^2 == max(n^2,eps^2)).
"""

import numpy as np
from contextlib import ExitStack

import ml_dtypes

import concourse.bass as bass
import concourse.bacc as bacc
import concourse.mybir as mybir
import concourse.tile as tile
from concourse.bass_utils import run_bass_kernel_spmd

AF = mybir.ActivationFunctionType
ALU = mybir.AluOpType
F32 = mybir.dt.float32
F32R = mybir.dt.float32r
BF16 = mybir.dt.bfloat16
F8 = mybir.dt.float8e4
BF = ml_dtypes.bfloat16
NP8 = ml_dtypes.float8_e4m3
WSCALE = 256.0   # fp8 weights are scaled x256; de-scaled in the ACT port

B, D, K = 16384, 512, 11
NCORES = 8
BC = B // NCORES          # 2048 rows per core
NB = 512                  # batch tile
NT = BC // NB             # 4 batch tiles per core
HB = D // 128             # 4 feature blocks of 128
NV = K + 1                # 11 transform views + x itself
XV = K                    # view index of x
# pair r: (K, k) = pos_k for k<11 ; then (l, k) l<k = S[l,k]
PAIRS = [(K, k) for k in range(K)] + [
    (l, k) for l in range(K) for k in range(l + 1, K)
]
NPAIR = len(PAIRS)        # 66

# pair index of S[l,k] (l<k)
_S_IDX = {}
_r = K
for _l in range(K):
    for _k in range(_l + 1, K):
        _S_IDX[(_l, _k)] = _r
        _r += 1
# pairs that become computable once transform view v finishes
# (x view XV is always computed first): (r, view_a, view_b)
READY = {
    v: [(v, XV, v)] + [(_S_IDX[(l, v)], l, v) for l in range(v)]
    for v in range(K)
}
# contiguous groups for the rn_a*rn_b scaling: (a_view, b_lo, b_hi, r_lo)
GROUPS = [(XV, 0, K, 0)]
_r = K
for _l in range(K):
    _n = K - _l - 1
    if _n:
        GROUPS.append((_l, _l + 1, K, _r))
        _r += _n


def _sel_matrix() -> np.ndarray:
    """selc[r, kk] = 1 if pair r contributes to denominator kk."""
    sel = np.zeros((NPAIR, K), np.float32)
    for r, (a, b) in enumerate(PAIRS):
        if a == K:
            sel[r, b] = 1.0       # pos_k only in denominator k
        else:
            sel[r, a] = 1.0       # S[l,k] symmetric: denominators l and k
            sel[r, b] = 1.0
    return sel


def _build_program():
    nc = bacc.Bacc("TRN2", target_bir_lowering=False, debug=False)

    xT = nc.declare_dram_parameter("xT", [HB, 128, BC], F8, False)
    tw = nc.declare_dram_parameter("tw", [K, 3, HB, 128, D], F8, False)
    ew12 = nc.declare_dram_parameter("ew12", [2, HB, 128, D], F8, False)
    ew3 = nc.declare_dram_parameter("ew3", [HB, 128, D], BF16, False)
    selc = nc.declare_dram_parameter("selc", [NPAIR, K], F32, False)
    ident = nc.declare_dram_parameter("ident", [128, 128], BF16, False)
    y = nc.declare_dram_parameter("y", [NT, 1, NB], F32, True)

    with tile.TileContext(nc) as tc, ExitStack() as ctx:
        const = ctx.enter_context(tc.tile_pool(name="const", bufs=1))
        wstr = ctx.enter_context(tc.tile_pool(name="wstr", bufs=2))
        xpool = ctx.enter_context(tc.tile_pool(name="xpool", bufs=2))
        hpool = ctx.enter_context(tc.tile_pool(name="hpool", bufs=2))
        ypool = ctx.enter_context(tc.tile_pool(name="ypool", bufs=16))
        spool = ctx.enter_context(tc.tile_pool(name="spool", bufs=2))
        # psMM: layer matmuls (drained fast by ACT gelu/identity).
        # psZ: z3 groups (drained by DVE bn_stats + ACT copy) — separate
        # pool so a DVE dot burst can't stall the PE's layer pipeline.
        psMM = ctx.enter_context(tc.tile_pool(name="psMM", bufs=2,
                                              space="PSUM"))
        psZ = ctx.enter_context(tc.tile_pool(name="psZ", bufs=2,
                                             space="PSUM"))
        psT = ctx.enter_context(tc.tile_pool(name="psT", bufs=1,
                                             space="PSUM"))

        # ---- constants / resident weights ----
        ew_sb = []
        for layer in range(2):
            w = const.tile([128, HB, D], F8, name=f"ew{layer}")
            for ib in range(HB):
                nc.sync.dma_start(w[:, ib, :], ew12[layer, ib])
            ew_sb.append(w)
        ew3_sb = const.tile([128, HB, D], BF16, name="ew3_sb")
        for ib in range(HB):
            nc.sync.dma_start(ew3_sb[:, ib, :], ew3[ib])
        sel_sb = const.tile([NPAIR, K], F32R, name="sel_sb")
        nc.sync.dma_start(sel_sb[:], selc[:].bitcast(F32R))
        id_sb = const.tile([128, 128], BF16, name="id_sb")
        nc.sync.dma_start(id_sb[:], ident[:])
        ones11 = const.tile([K, 1], BF16, name="ones11")
        nc.vector.memset(ones11[:], 1.0)
        neg11 = const.tile([K, 1], BF16, name="neg11")
        nc.vector.memset(neg11[:], -1.0)

        def mlp_fp8(in3, w3, wrow, name, out_dtype):
            """fp8 DoubleRow layer, biases all zero (guaranteed by the
            fast-path gate). in3 [128, HB, NB] fp8; w3 [128, *, D] fp8
            scaled x256 (de-scaled via the ACT scale port). Gelu runs on
            merged jb-pairs ([128, 1024]) to halve ACT dispatch overhead.
            """
            out_sb = hpool.tile([128, HB, NB], out_dtype, name=name)
            for jp in range(2):
                ps = psMM.tile([128, 2, NB], F32, name="mm")
                for jb2 in range(2):
                    jb = 2 * jp + jb2
                    for p in range(2):
                        nc.tensor.matmul(
                            ps[:, jb2, :],
                            w3[:, wrow + 2 * p:wrow + 2 * p + 2,
                               jb * 128:(jb + 1) * 128],
                            in3[:, 2 * p:2 * p + 2, :],
                            start=(p == 0), stop=(p == 1),
                            perf_mode=mybir.MatmulPerfMode.DoubleRow,
                        )
                nc.scalar.activation(out_sb[:, 2 * jp:2 * jp + 2, :], ps[:],
                                     AF.Gelu, scale=1.0 / WSCALE)
            return out_sb

        def encode(y0s, ssum, qsum, e1, v):
            e2 = mlp_fp8(e1, ew_sb[1], 0, "e2", BF16)
            # z3 emitted sample-major (bf16 matmul for precision), stored
            # RAW (uncentered): the mean-centering folds into the dot
            # corrections via dot(za-ua,zb-ub) = dot(za,zb) - 512*ua*ub;
            # sums/sumsq ride the ACT accumulator for free.
            y0 = ypool.tile([128, HB, NB], BF16, name="y0")
            for sb in range(HB):
                ps = psZ.tile([128, NB], F32, name="zz")
                for ib in range(HB):
                    nc.tensor.matmul(
                        ps[:],
                        e2[:, ib, sb * 128:(sb + 1) * 128],
                        ew3_sb[:, ib, :],
                        start=(ib == 0), stop=(ib == HB - 1),
                    )
                scrz = spool.tile([128, NB], BF16, name="scrz", bufs=2)
                c = sb * NV + v
                if (v * HB + sb) % 5 == 4:
                    # DVE variant: copy+accum, then square via ps * y0_bf16
                    # (DVE may read only one PSUM input) — keeps the ACT
                    # and DVE loads balanced
                    nc.vector.tensor_scalar(
                        y0[:, sb, :], ps[:], 0.0, 0.0, ALU.add,
                        ALU.add, accum_out=ssum[:, c:c + 1])
                    nc.vector.scalar_tensor_tensor(
                        scrz[:], ps[:], 0.0, y0[:, sb, :],
                        ALU.add, ALU.mult,
                        accum_out=qsum[:, c:c + 1])
                else:
                    nc.scalar.activation(y0[:, sb, :], ps[:], AF.Identity,
                                         accum_out=ssum[:, c:c + 1])
                    nc.scalar.activation(scrz[:], ps[:], AF.Square,
                                         accum_out=qsum[:, c:c + 1])
            y0s[v] = y0

        def fire_dots(y0s, dts, v):
            for (r, a, b) in READY[v]:
                # NOTE: tensor_tensor_reduce faults trn2 hw here; the
                # equivalent scalar_tensor_tensor + accum_out works.
                for sb in range(HB):
                    scr = spool.tile([128, NB], BF16, name="scr", bufs=2)
                    nc.vector.scalar_tensor_tensor(
                        scr[:], y0s[a][:, sb, :], 0.0,
                        y0s[b][:, sb, :], ALU.add, ALU.mult,
                        accum_out=dts[sb][:, r:r + 1])

        def tail1(st):
            # norm factors for a finished tile: 512*var = qsum-ssum^2/512;
            # m512 = -ssum/512 so the pair correction -512*mu_a*mu_b =
            # ssum_b * m512_a.
            t_idx, dts, ssum, qsum = st
            m512 = spool.tile([128, HB * NV], F32, name="m512")
            nc.vector.tensor_scalar_mul(m512[:], ssum[:], -1.0 / 512.0)
            t48 = spool.tile([128, HB * NV], F32, name="t48")
            nc.vector.scalar_tensor_tensor(t48[:], ssum[:], 0.0, m512[:],
                                           ALU.add, ALU.mult)
            nc.vector.scalar_tensor_tensor(t48[:], t48[:], 0.0, qsum[:],
                                           ALU.add, ALU.add)
            nc.vector.tensor_scalar_max(t48[:], t48[:], 1e-16)
            s48 = spool.tile([128, HB * NV], F32, name="s48")
            nc.scalar.activation(s48[:], t48[:], AF.Sqrt)
            rn48 = spool.tile([128, HB * NV], F32, name="rn48")
            nc.vector.reciprocal(rn48[:], s48[:])
            return m512, rn48

        def tail2(st, m512, rn48):
            # scale, transpose, logsumexp, loss
            t_idx, dts, ssum, qsum = st
            dp = spool.tile([128, HB, NPAIR], BF16, name="dp")
            expd = spool.tile([NPAIR, 4 * 128], F32R, name="expd")
            pos_sb = spool.tile([K, 4 * 128], BF16, name="pos_sb")
            for sb in range(HB):
                o = sb * NV
                for (a, blo, bhi, rlo) in GROUPS:
                    n = bhi - blo
                    # mean-fold correction: D -= 512 * mu_a * mu_b
                    nc.vector.scalar_tensor_tensor(
                        dts[sb][:, rlo:rlo + n],
                        ssum[:, o + blo:o + bhi],
                        m512[:, o + a:o + a + 1],
                        dts[sb][:, rlo:rlo + n],
                        ALU.mult, ALU.add)
                    nc.vector.scalar_tensor_tensor(
                        dp[:, sb, rlo:rlo + n], dts[sb][:, rlo:rlo + n],
                        rn48[:, o + a:o + a + 1], rn48[:, o + blo:o + bhi],
                        ALU.mult, ALU.mult)
                pst = psT.tile([NPAIR, 128], BF16, name="pst")
                nc.tensor.matmul(pst[:], dp[:, sb, :], id_sb[:],
                                 is_transpose=True)
                nc.scalar.activation(expd[:, sb * 128:(sb + 1) * 128],
                                     pst[:], AF.Exp)
                nc.vector.tensor_copy(pos_sb[:, sb * 128:(sb + 1) * 128],
                                      pst[0:K, :])
            ps_den = psT.tile([K, NB], F32, name="den")
            nc.tensor.matmul(ps_den[:], sel_sb[:], expd[:],
                             start=True, stop=True)
            ld = spool.tile([K, NB], BF16, name="ld")
            nc.scalar.activation(ld[:], ps_den[:], AF.Ln)
            ps_loss = psT.tile([K, NB], F32, name="den")[0:1, :]
            nc.tensor.matmul(ps_loss, ones11[:], ld[:],
                             start=True, stop=False)
            nc.tensor.matmul(ps_loss, neg11[:], pos_sb[:],
                             start=False, stop=True)
            loss_sb = spool.tile([1, NB], F32, name="loss_sb")
            nc.vector.tensor_copy(loss_sb[:], ps_loss)
            nc.sync.dma_start(y[t_idx], loss_sb[:])

        # ---- main loop over batch tiles. The lse tail (tail2) of tile t
        # is emitted inside tile t+1 after its first transform view: all
        # of its inputs are complete by then, so its Exp/Ln don't
        # head-of-line block the next tile's gelus in the ACT FIFO (a
        # measured ~30us ACT stall per tile boundary when emitted
        # inline). Dots stay fully inline — deferring them starves the
        # DVE (measured). ----
        pend = None
        for t in range(NT):
            x_sb = xpool.tile([128, HB, NB], F8, name="x_sb")
            for hb in range(HB):
                nc.sync.dma_start(x_sb[:, hb, :],
                                  xT[hb, :, t * NB:(t + 1) * NB])

            ssum = spool.tile([128, HB * NV], F32, name="ssum")
            qsum = spool.tile([128, HB * NV], F32, name="qsum")
            dts = [spool.tile([128, NPAIR], F32, name="dt", bufs=8)
                   for _ in range(HB)]
            y0s = [None] * NV

            e1x = mlp_fp8(x_sb, ew_sb[0], 0, "e1", F8)
            encode(y0s, ssum, qsum, e1x, XV)
            for k in range(K):
                tw_sb = wstr.tile([128, 3 * HB, D], F8, name="tw_sb")
                for layer in range(3):
                    for ib in range(HB):
                        nc.sync.dma_start(tw_sb[:, layer * HB + ib, :],
                                          tw[k, layer, ib])
                h1 = mlp_fp8(x_sb, tw_sb, 0, "h1", F8)
                h2 = mlp_fp8(h1, tw_sb, HB, "h2", F8)
                # transform L3 is linear and feeds encoder L1 (also linear
                # pre-gelu): both are fused host-side into W3f = tW3 @ eW1,
                # b3f = tb3 @ eW1 + eb1 — one layer instead of two.
                e1k = mlp_fp8(h2, tw_sb, 2 * HB, "e1", F8)
                encode(y0s, ssum, qsum, e1k, k)
                fire_dots(y0s, dts, k)
                if k == 0 and pend is not None:
                    tail2(*pend)
                    pend = None
            st = (t, dts, ssum, qsum)
            m512, rn48 = tail1(st)
            pend = (st, m512, rn48)
        tail2(*pend)

    nc.compile()
    return nc


_NC_CACHE = None


def _get_program():
    global _NC_CACHE
    if _NC_CACHE is None:
        _NC_CACHE = _build_program()
    return _NC_CACHE


def _make_in_maps(inputs):
    f = lambda a: np.ascontiguousarray(np.asarray(a, np.float32))

    def pack_w(a):  # [*, 512 in, 512 out] -> [*, HB, 128, out] bf16
        a = f(a)
        return np.ascontiguousarray(
            a.reshape(a.shape[:-2] + (HB, 128, D)).astype(BF))

    def pack_b(a):  # [K, 512] -> [128, K*HB]
        return np.ascontiguousarray(
            f(a).reshape(K, HB, 128).transpose(2, 0, 1).reshape(128, K * HB))

    def pack_w8(a):  # scaled x256, fp8 e4m3
        a = f(a) * WSCALE
        return np.ascontiguousarray(
            a.reshape(a.shape[:-2] + (HB, 128, D)).astype(NP8))

    # fuse transform L3 into encoder L1 (both linear pre-gelu):
    # e1_k = gelu(h2 @ (tW3_k @ eW1) + (tb3_k @ eW1 + eb1))
    eW1f = f(inputs["eW1"])
    tW3f = np.einsum("kij,jh->kih", f(inputs["tW3"]), eW1f)
    tb3f = f(inputs["tb3"]) @ eW1f + f(inputs["eb1"])[None, :]
    tw_full = np.ascontiguousarray(np.stack(
        [pack_w8(inputs["tW1"]), pack_w8(inputs["tW2"]), pack_w8(tW3f)],
        axis=1))                                     # [K, 3, HB, 128, D]
    ew12_full = np.ascontiguousarray(np.stack(
        [pack_w8(inputs["eW1"]), pack_w8(inputs["eW2"])],
        axis=0))                                     # [2, HB, 128, D]
    shared = {
        "tw": tw_full,
        "ew12": ew12_full,
        "ew3": pack_w(inputs["eW3"]),
        "selc": _sel_matrix(),
        "ident": np.eye(128, dtype=BF),
    }
    xT_full = np.ascontiguousarray(f(inputs["x"]).T)  # [512, 16384]
    in_maps = []
    for i in range(NCORES):
        m = dict(shared)
        m["xT"] = np.ascontiguousarray(
            xT_full[:, i * BC:(i + 1) * BC]).reshape(HB, 128, BC).astype(NP8)
        in_maps.append(m)
    return in_maps


def _fast_ok(inputs):
    zeros = ("ln_b", "eb1", "eb2", "eb3", "tb1", "tb2", "tb3")
    return (np.allclose(np.asarray(inputs["ln_g"], np.float32), 1.0)
            and all(np.allclose(np.asarray(inputs[z], np.float32), 0.0)
                    for z in zeros))


def _numpy_fallback(inputs):
    """Exact fallback for inputs outside the fast-path assumptions."""
    f = lambda a: np.asarray(a, np.float64)
    x = f(inputs["x"])

    def _erf(z):
        try:
            from scipy.special import erf
            return erf(z)
        except ImportError:
            import math
            return np.vectorize(math.erf)(z)

    gelu = lambda h: 0.5 * h * (1.0 + _erf(h / np.sqrt(2.0)))

    def layernorm(h, g, b, eps=1e-5):
        mu = h.mean(-1, keepdims=True)
        var = h.var(-1, keepdims=True)
        return (h - mu) / np.sqrt(var + eps) * g + b

    def encoder(h):
        h = gelu(h @ f(inputs["eW1"]) + f(inputs["eb1"]))
        h = gelu(h @ f(inputs["eW2"]) + f(inputs["eb2"]))
        h = h @ f(inputs["eW3"]) + f(inputs["eb3"])
        return layernorm(h, f(inputs["ln_g"]), f(inputs["ln_b"]))

    def normalize(v):
        n = np.sqrt((v * v).sum(-1, keepdims=True))
        return v / np.maximum(n, 1e-8)

    h = gelu(np.einsum("bi,kij->kbj", x, f(inputs["tW1"]))
             + f(inputs["tb1"])[:, None, :])
    h = gelu(np.einsum("kbi,kij->kbj", h, f(inputs["tW2"]))
             + f(inputs["tb2"])[:, None, :])
    tx = (np.einsum("kbi,kij->kbj", h, f(inputs["tW3"]))
          + f(inputs["tb3"])[:, None, :])
    z = encoder(x)
    zk = encoder(tx)
    zn = normalize(z)
    zkn = normalize(zk)
    pos = np.einsum("bh,kbh->kb", zn, zkn)
    S = np.einsum("lbh,kbh->lkb", zkn, zkn)
    diag = np.eye(K, dtype=bool)[:, :, None]
    Sm = np.where(diag, -np.inf, S)
    allt = np.concatenate([pos[None], Sm], axis=0)
    mx = allt.max(axis=0)
    log_den = mx + np.log(np.exp(allt - mx).sum(axis=0))
    return (-(pos - log_den).sum(axis=0)).astype(np.float32)


def run(inputs, trace=False):
    nc = _get_program()
    res = run_bass_kernel_spmd(nc, _make_in_maps(inputs),
                               list(range(NCORES)), trace=trace)
    out = np.concatenate([res.results[i]["y"].reshape(BC)
                          for i in range(NCORES)])
    return out.astype(np.float32), res


def kernel(**inputs):
    if not _fast_ok(inputs):
        return _numpy_fallback(inputs)
    out, _ = run(inputs)
    return out
